# revision 14
# baseline (speedup 1.0000x reference)
"""Trainium2 Bass kernel for nn_CrossMed4 (CrossMed-style GRU-over-GRU model).

Strategy (8 NeuronCores, data-parallel over the patient batch B=16 -> 2/core):
- Embedding lookups via bulk SWDGE dma_gather in token-on-partition layout,
  in bf16 (256B rows). The four monitor token streams are spread across the
  four SWDGE queue contexts (each queue = its own Q7 core pair + DMA rings),
  so descriptor generation for the streams runs 4-way concurrent instead of
  serialized on queue 0.
- The sum-over-codes reduction AND the transpose into [D, group] layout happen
  in ONE PE matmul per rank against a constant 0/1 summing matrix S5.
- Both GRU levels run in transposed layout (H^T [D=128, batch*keys]) in bf16;
  input-gate projections and bias adds are folded into PE matmuls.
- r/z gate preactivations accumulate in PSUM (fp32) on an ACT-copied xg
  preload; sigmoid/tanh on ACT; remaining elementwise on DVE in bf16.
"""
import numpy as np
import ml_dtypes

try:
    import concourse.bass as bass  # noqa: F401
except ImportError:
    import sys
    sys.path.insert(0, "/opt/trn_rl_repo")

import concourse.bacc as bacc
import concourse.bass as bass
import concourse.mybir as mybir
import concourse.tile as tile
from concourse.bass_utils import run_bass_kernel_spmd

F32 = mybir.dt.float32
BF16 = mybir.dt.bfloat16
I16 = mybir.dt.int16
NPBF = ml_dtypes.bfloat16

B, V, M, L, D, OUT = 16, 16, 32, 24, 128, 193
VOCAB = {"cond": 5000, "proc": 2000, "drug": 600, "lab_item": 700,
         "lab_value": 200, "inj_item": 400, "inj_value": 200}
NCORES = 8
BL = B // NCORES            # 2 patients per core
NBV = BL * V                # 32 visit groups
TCH = 4                     # monitor steps per chunk
NCHUNK = M // TCH           # 8
GC = NBV * TCH              # 128 groups per chunk
RC = (GC + 4) // 5          # 26 ranks per chunk
IDXC = RC * 128             # 3328 idxs per chunk per stream
VRANKS = (NBV + 4) // 5     # 7
VIDX = VRANKS * 128         # 896

# SWDGE queue assignment: one Q7 core pair per queue, 4 queues exist.
QMAP = {"cond": 0, "proc": 1, "drug": 2,
        "lab_item": 0, "lab_value": 1, "inj_item": 2, "inj_value": 3}


# --------------------------------------------------------------------------
# host-side index / weight packing
# --------------------------------------------------------------------------

def _wrap_idx(flat):
    # token i lives at [i % 16, i // 16]; the gather ucode's Q7 cores each
    # read their own 16-partition band, so replicate to all 8 bands.
    n = flat.shape[0]
    return np.tile(flat.reshape(n // 16, 16).T, (8, 1)).astype(np.int16)


def _build_monitor_idx(tok):
    """tok [BL, V, M, L] -> wrapped [128, NCHUNK*IDXC//16] int16."""
    flat = np.zeros(NCHUNK * IDXC, dtype=np.int64)
    t = np.asarray(tok)
    for c in range(NCHUNK):
        base = c * IDXC
        for r in range(RC):
            for j in range(5):
                slot = 5 * r + j
                if slot >= GC:
                    continue
                mi, rem = divmod(slot, NBV)
                b, v = divmod(rem, V)
                flat[base + r * 128 + j * 24: base + r * 128 + j * 24 + 24] = \
                    t[b, v, c * TCH + mi, :]
    return _wrap_idx(flat)


def _build_visit_idx(tok):
    flat = np.zeros(VIDX, dtype=np.int64)
    t = np.asarray(tok)
    for r in range(VRANKS):
        for j in range(5):
            slot = 5 * r + j
            if slot >= NBV:
                continue
            b, v = divmod(slot, V)
            flat[r * 128 + j * 24: r * 128 + j * 24 + 24] = t[b, v, :]
    return _wrap_idx(flat)


def _prep_shared(inputs):
    """Weight repacking shared by all cores (pure layout transforms)."""
    f = {k: np.asarray(v, dtype=np.float32) for k, v in inputs.items()
         if not k.startswith("tok_")}
    sh = {}
    mwih, mwhh = f["mgru_wih"], f["mgru_whh"]
    mbih, mbhh = f["mgru_bih"], f["mgru_bhh"]
    vwih, vwhh = f["vgru_wih"], f["vgru_whh"]
    vbih, vbhh = f["vgru_bih"], f["vgru_bhh"]

    def packT(w_keys):  # [K, 3D, D] -> [128, K*3*128], col (k*3+gi)*128+gu
        k = w_keys.shape[0]
        out = np.zeros((128, k * 3 * 128), dtype=np.float32)
        for ki in range(k):
            for gi in range(3):
                out[:, (ki * 3 + gi) * 128:(ki * 3 + gi + 1) * 128] = \
                    w_keys[ki, gi * 128:(gi + 1) * 128, :].T
        return out

    def pack_xgb(bih, bhh, keys):  # -> [1, len(keys)*384]
        rows = []
        for k in keys:
            b = bih[k].copy()
            b[:2 * D] += bhh[k][:2 * D]
            rows.append(b)
        return np.concatenate(rows)[None, :].astype(np.float32)

    sh["mwhhT"] = packT(mwhh)
    sh["mwihT34"] = packT(mwih[3:5])
    sh["mxgb34"] = pack_xgb(mbih, mbhh, [3, 4])
    sh["mwihT012"] = packT(mwih[0:3])
    sh["mxgb012"] = pack_xgb(mbih, mbhh, [0, 1, 2])
    sh["bhn_bc"] = np.repeat(mbhh[:, 2 * D:].T, NBV, axis=1).astype(np.float32)
    sh["vwhhT"] = packT(vwhh)
    sh["vwihT04"] = packT(vwih[0:5])
    sh["vxgb04"] = pack_xgb(vbih, vbhh, [0, 1, 2, 3, 4])
    u_rows, c_rows = [], []
    for k in (5, 6):
        u_rows.append(vwih[k] @ f["info_w"][k - 5])
        cv = vwih[k] @ f["info_b"][k - 5] + vbih[k]
        cv[:2 * D] += vbhh[k][:2 * D]
        c_rows.append(cv)
    sh["vxg56u"] = np.concatenate(u_rows)[None, :].astype(np.float32)
    sh["vxg56c"] = np.concatenate(c_rows)[None, :].astype(np.float32)
    sh["vbhn_bc"] = np.repeat(vbhh[:, 2 * D:].T, BL, axis=1).astype(np.float32)
    s5 = np.zeros((128, 5), dtype=np.float32)
    for j in range(5):
        s5[j * 24:(j + 1) * 24, j] = 1.0
    sh["S5"] = s5
    sh["ones"] = np.ones((1, 224), dtype=np.float32)
    fcw = np.zeros((128, 7 * OUT), dtype=np.float32)
    for k in range(7):
        fcw[:, k * OUT:(k + 1) * OUT] = f["fc_w"][k * D:(k + 1) * D, :]
    sh["fcw"] = fcw
    sh["fcb"] = f["fc_b"][None, :].astype(np.float32)
    for name in VOCAB:
        sh["emb_" + name] = f["emb_" + name]
    # everything except the fp32 DVE-side biases goes to bf16
    for k in list(sh.keys()):
        if k not in ("bhn_bc", "vbhn_bc"):
            sh[k] = sh[k].astype(NPBF)
    return sh


def _prep_core(inputs, shared, core):
    b0 = core * BL
    m = dict(shared)
    for name in ("cond", "proc", "drug"):
        m["idx_" + name] = _build_visit_idx(
            np.asarray(inputs["tok_" + name])[b0:b0 + BL])
    for name in ("lab_item", "lab_value", "inj_item", "inj_value"):
        m["idx_" + name] = _build_monitor_idx(
            np.asarray(inputs["tok_" + name])[b0:b0 + BL])
    wa = np.zeros((1, 64), dtype=np.float32)
    wa[0, :NBV] = np.asarray(inputs["weight"], np.float32)[b0:b0 + BL].reshape(NBV)
    wa[0, NBV:] = np.asarray(inputs["age"], np.float32)[b0:b0 + BL].reshape(NBV)
    m["wa"] = wa.astype(NPBF)
    return m


# --------------------------------------------------------------------------
# device program
# --------------------------------------------------------------------------

CONST_SHAPES = (("mwhhT", [128, 1920], BF16), ("mwihT34", [128, 768], BF16),
                ("mxgb34", [1, 768], BF16), ("mwihT012", [128, 1152], BF16),
                ("mxgb012", [1, 1152], BF16), ("bhn_bc", [128, 160], F32),
                ("vwhhT", [128, 2688], BF16), ("vwihT04", [128, 1920], BF16),
                ("vxgb04", [1, 1920], BF16), ("vxg56u", [1, 768], BF16),
                ("vxg56c", [1, 768], BF16), ("vbhn_bc", [128, 14], F32),
                ("S5", [128, 5], BF16), ("ones", [1, 224], BF16),
                ("wa", [1, 64], BF16), ("fcw", [128, 7 * OUT], BF16),
                ("fcb", [1, OUT], BF16))


def build_nc(stage="full"):
    nc = bacc.Bacc("TRN2", target_bir_lowering=False, debug=False,
                   num_devices=NCORES, num_swdge_queues=4)
    dt = {}
    for name, voc in VOCAB.items():
        dt["emb_" + name] = nc.dram_tensor("emb_" + name, [voc, D], BF16,
                                           kind="ExternalInput")
    for name in ("cond", "proc", "drug"):
        dt["idx_" + name] = nc.dram_tensor("idx_" + name, [128, VIDX // 16],
                                           I16, kind="ExternalInput")
    for name in ("lab_item", "lab_value", "inj_item", "inj_value"):
        dt["idx_" + name] = nc.dram_tensor("idx_" + name,
                                           [128, NCHUNK * IDXC // 16], I16,
                                           kind="ExternalInput")
    for name, shape, dty in CONST_SHAPES:
        dt[name] = nc.dram_tensor(name, shape, dty, kind="ExternalInput")
    out_logits = nc.dram_tensor("logits", [BL, OUT], F32, kind="ExternalOutput")

    with tile.TileContext(nc) as tc:
        _program(nc, tc, dt, out_logits, stage)
    nc.compile()
    return nc


def _program(nc, tc, dt, out_logits, stage="full"):
    STAGES = ("consts", "visit", "chunks", "gru", "vgru", "full")
    lvl = STAGES.index(stage)
    import contextlib
    ctx = contextlib.ExitStack()
    with ctx:
        cpool = ctx.enter_context(tc.tile_pool(name="const", bufs=1))
        gpool = ctx.enter_context(tc.tile_pool(name="gather", bufs=2))
        spool = ctx.enter_context(tc.tile_pool(name="work", bufs=2))
        xgpool = ctx.enter_context(tc.tile_pool(name="xg34", bufs=3))
        hpool = ctx.enter_context(tc.tile_pool(name="h", bufs=2))
        ppool = ctx.enter_context(tc.tile_pool(name="psum", bufs=2,
                                               space="PSUM"))

        # ---- load constants to SBUF
        cb = {}
        for name, shape, dty in CONST_SHAPES:
            t = cpool.tile(shape, dty, tag=name)
            nc.sync.dma_start(t[:], dt[name].ap())
            cb[name] = t
        idx = {}
        for name in ("cond", "proc", "drug", "lab_item", "lab_value",
                     "inj_item", "inj_value"):
            shape = [128, VIDX // 16] if name in ("cond", "proc", "drug") \
                else [128, NCHUNK * IDXC // 16]
            t = cpool.tile(shape, I16, tag="idx_" + name)
            nc.sync.dma_start(t[:], dt["idx_" + name].ap())
            idx[name] = t

        S5 = cb["S5"]
        ones = cb["ones"]

        if lvl < 1:
            lg = spool.tile([BL, OUT], F32, tag="lg")
            nc.scalar.copy(lg[:], cb["fcw"][0:BL, 0:OUT])
            nc.sync.dma_start(out_logits.ap(), lg[:])
            return

        # ---- visit-level features eT[k] = [128, 32]
        vgt = {}
        for name in ("cond", "proc", "drug"):
            gt = gpool.tile([128, VIDX], BF16, tag="vg_" + name)
            nc.gpsimd.dma_gather(
                gt[:].rearrange("p (r e) -> p r e", e=D),
                dt["emb_" + name].ap(), idx[name][:], VIDX, VIDX, D,
                queue_num=QMAP[name])
            vgt[name] = gt
        eT = {}
        for name in ("cond", "proc", "drug"):
            gt = vgt[name]
            pr = ppool.tile([128, VRANKS * 5], F32, tag="red")
            for r in range(VRANKS):
                nc.tensor.matmul(pr[:, 5 * r:5 * r + 5],
                                 gt[:, r * D:(r + 1) * D], S5[:],
                                 start=True, stop=True)
            et = cpool.tile([128, NBV], BF16, tag="eT_" + name)
            nc.scalar.copy(et[:], pr[:, :NBV])
            eT[name] = et

        # ---- XGc for monitor keys 0-2: [128, 3*96], col gi*96 + k*32 + bv
        xgc = cpool.tile([128, 288], BF16, tag="xgc")
        for k, name in enumerate(("cond", "proc", "drug")):
            pk = ppool.tile([128, 96], F32, tag="xg")
            for gi in range(3):
                off = (k * 3 + gi) * 128
                nc.tensor.matmul(pk[:, gi * 32:(gi + 1) * 32],
                                 cb["mwihT012"][:, off:off + 128],
                                 eT[name][:], start=True, stop=False)
                nc.tensor.matmul(pk[:, gi * 32:(gi + 1) * 32],
                                 cb["mxgb012"][0:1, off:off + 128],
                                 ones[0:1, 0:NBV], start=False, stop=True)
            nc.scalar.copy(
                xgc[:].rearrange("p (g c) -> p g c", g=3)[:, :, k * 32:(k + 1) * 32],
                pk[:].rearrange("p (g c) -> p g c", g=3))

        if lvl < 2:
            lg = spool.tile([BL, OUT], F32, tag="lg")
            nc.scalar.copy(lg[:], xgc[0:BL, 0:OUT])
            nc.sync.dma_start(out_logits.ap(), lg[:])
            return

        # ---- monitor chunks + GRU
        h_prev = hpool.tile([128, 160], BF16, tag="H")
        nc.vector.memset(h_prev[:], 0.0)

        STREAMS = ("lab_item", "lab_value", "inj_item", "inj_value")
        for c in range(NCHUNK):
            # issue all 4 streams' sub-gathers round-robin across queues
            gts = {}
            for name in STREAMS:
                gts[name] = gpool.tile([128, IDXC], BF16, tag=name,
                                       name="g_" + name)
            # dma_gather is capped at 1024 idxs (idx-read free dim <= 64),
            # so split each chunk into <=8-rank sub-gathers.
            for r0 in range(0, RC, 8):
                nr = min(8, RC - r0)
                for name in STREAMS:
                    nc.gpsimd.dma_gather(
                        gts[name][:].rearrange("p (r e) -> p r e", e=D)
                        [:, r0:r0 + nr, :],
                        dt["emb_" + name].ap(),
                        idx[name][:, c * (IDXC // 16) + r0 * 8:
                                  c * (IDXC // 16) + r0 * 8 + nr * 8],
                        nr * 128, nr * 128, D,
                        queue_num=QMAP[name])

            xg34 = xgpool.tile([128, 768], BF16, tag="xg34")
            for k, iname, vname in ((3, "lab_item", "lab_value"),
                                    (4, "inj_item", "inj_value")):
                it, vt = gts[iname], gts[vname]
                nc.vector.tensor_tensor(it[:], it[:], vt[:],
                                        op=mybir.AluOpType.mult)
                pr = ppool.tile([128, RC * 5], F32, tag="red")
                for r in range(RC):
                    nc.tensor.matmul(pr[:, 5 * r:5 * r + 5],
                                     it[:, r * D:(r + 1) * D], S5[:],
                                     start=True, stop=True)
                labT = spool.tile([128, GC], BF16, tag="pairT")
                nc.scalar.copy(labT[:], pr[:, :GC])
                pxg = ppool.tile([128, 384], F32, tag="xg")
                for gi in range(3):
                    off = ((k - 3) * 3 + gi) * 128
                    nc.tensor.matmul(pxg[:, gi * 128:(gi + 1) * 128],
                                     cb["mwihT34"][:, off:off + 128],
                                     labT[:], start=True, stop=False)
                    nc.tensor.matmul(pxg[:, gi * 128:(gi + 1) * 128],
                                     cb["mxgb34"][0:1, off:off + 128],
                                     ones[0:1, 0:128], start=False, stop=True)
                # copy into xg34: col gi*256 + mi*64 + (k-3)*32 + bv
                for gi in range(3):
                    nc.scalar.copy(
                        xg34[:].rearrange("p (g m kb) -> p g m kb", g=3, m=TCH)
                        [:, gi, :, (k - 3) * 32:(k - 2) * 32],
                        pxg[:].rearrange("p (g m b) -> p g m b", g=3, m=TCH)
                        [:, gi, :, :])

            for mi in range(TCH if lvl >= 3 else 0):
                prz = ppool.tile([128, 320], F32, tag="prz")
                # xg preload: r/z const + per-step slices
                nc.scalar.copy(
                    prz[:].rearrange("p (g c) -> p g c", g=2)[:, :, 0:96],
                    xgc[:].rearrange("p (g c) -> p g c", g=3)[:, 0:2, :])
                nc.scalar.copy(
                    prz[:].rearrange("p (g c) -> p g c", g=2)[:, :, 96:160],
                    xg34[:].rearrange("p (g c) -> p g c", g=3)
                    [:, 0:2, mi * 64:(mi + 1) * 64])
                pn = ppool.tile([128, 160], F32, tag="pn")
                for k in range(5):
                    hs = h_prev[:, k * 32:(k + 1) * 32]
                    for gi in range(2):
                        nc.tensor.matmul(
                            prz[:, gi * 160 + k * 32: gi * 160 + (k + 1) * 32],
                            cb["mwhhT"][:, (k * 3 + gi) * 128:(k * 3 + gi + 1) * 128],
                            hs, start=False, stop=True, skip_group_check=True)
                    nc.tensor.matmul(
                        pn[:, k * 32:(k + 1) * 32],
                        cb["mwhhT"][:, (k * 3 + 2) * 128:(k * 3 + 3) * 128],
                        hs, start=True, stop=True)
                rz = spool.tile([128, 320], BF16, tag="rz")
                nc.scalar.activation(rz[:], prz[:],
                                     mybir.ActivationFunctionType.Sigmoid)
                u = spool.tile([128, 160], BF16, tag="u")
                nc.vector.tensor_tensor(u[:], pn[:], cb["bhn_bc"][:],
                                        op=mybir.AluOpType.add)
                nc.vector.tensor_tensor(u[:], rz[:, 0:160], u[:],
                                        op=mybir.AluOpType.mult)
                npre = spool.tile([128, 160], BF16, tag="npre")
                nc.vector.tensor_tensor(
                    npre[:, 0:96], u[:, 0:96],
                    xgc[:].rearrange("p (g c) -> p g c", g=3)[:, 2, :],
                    op=mybir.AluOpType.add)
                nc.vector.tensor_tensor(
                    npre[:, 96:160], u[:, 96:160],
                    xg34[:, 2 * 256 + mi * 64: 2 * 256 + (mi + 1) * 64],
                    op=mybir.AluOpType.add)
                nt = spool.tile([128, 160], BF16, tag="nt")
                nc.scalar.activation(nt[:], npre[:],
                                     mybir.ActivationFunctionType.Tanh)
                t3 = spool.tile([128, 160], BF16, tag="t3")
                nc.vector.tensor_tensor(t3[:], h_prev[:], nt[:],
                                        op=mybir.AluOpType.subtract)
                nc.vector.tensor_tensor(t3[:], t3[:], rz[:, 160:320],
                                        op=mybir.AluOpType.mult)
                h_new = hpool.tile([128, 160], BF16, tag="H")
                nc.vector.tensor_tensor(h_new[:], t3[:], nt[:],
                                        op=mybir.AluOpType.add)
                h_prev = h_new

        if lvl < 4:
            lg = spool.tile([BL, OUT], F32, tag="lg")
            nc.vector.memset(lg[:], 0.0)
            src = h_prev if lvl >= 3 else xg34
            nc.scalar.copy(lg[:, 0:160], src[0:BL, 0:160])
            nc.sync.dma_start(out_logits.ap(), lg[:])
            return

        # ---- visit GRU
        vxg = cpool.tile([128, 672], BF16, tag="vxg")  # col gi*224 + k*32 + bv
        for k in range(5):
            pk = ppool.tile([128, 96], F32, tag="xg")
            for gi in range(3):
                off = (k * 3 + gi) * 128
                nc.tensor.matmul(pk[:, gi * 32:(gi + 1) * 32],
                                 cb["vwihT04"][:, off:off + 128],
                                 h_prev[:, k * 32:(k + 1) * 32],
                                 start=True, stop=False)
                nc.tensor.matmul(pk[:, gi * 32:(gi + 1) * 32],
                                 cb["vxgb04"][0:1, off:off + 128],
                                 ones[0:1, 0:NBV], start=False, stop=True)
            nc.scalar.copy(
                vxg[:].rearrange("p (g c) -> p g c", g=3)[:, :, k * 32:(k + 1) * 32],
                pk[:].rearrange("p (g c) -> p g c", g=3))
        for k in (5, 6):
            pk = ppool.tile([128, 96], F32, tag="xg")
            for gi in range(3):
                off = ((k - 5) * 3 + gi) * 128
                nc.tensor.matmul(pk[:, gi * 32:(gi + 1) * 32],
                                 cb["vxg56u"][0:1, off:off + 128],
                                 cb["wa"][0:1, (k - 5) * 32:(k - 4) * 32],
                                 start=True, stop=False)
                nc.tensor.matmul(pk[:, gi * 32:(gi + 1) * 32],
                                 cb["vxg56c"][0:1, off:off + 128],
                                 ones[0:1, 0:NBV], start=False, stop=True)
            nc.scalar.copy(
                vxg[:].rearrange("p (g c) -> p g c", g=3)[:, :, k * 32:(k + 1) * 32],
                pk[:].rearrange("p (g c) -> p g c", g=3))

        vh_prev = hpool.tile([128, 14], BF16, tag="VH")
        nc.vector.memset(vh_prev[:], 0.0)
        for v in range(V):
            prz = ppool.tile([128, 28], F32, tag="prz")
            nc.scalar.copy(
                prz[:].rearrange("p (g c) -> p g c", g=2),
                vxg[:].rearrange("p (g k b v2) -> p g k b v2", g=3, k=7, b=BL)
                [:, 0:2, :, :, v])
            pn = ppool.tile([128, 14], F32, tag="pn")
            for k in range(7):
                hs = vh_prev[:, k * 2:(k + 1) * 2]
                for gi in range(2):
                    nc.tensor.matmul(
                        prz[:, gi * 14 + k * 2: gi * 14 + (k + 1) * 2],
                        cb["vwhhT"][:, (k * 3 + gi) * 128:(k * 3 + gi + 1) * 128],
                        hs, start=False, stop=True, skip_group_check=True)
                nc.tensor.matmul(
                    pn[:, k * 2:(k + 1) * 2],
                    cb["vwhhT"][:, (k * 3 + 2) * 128:(k * 3 + 3) * 128],
                    hs, start=True, stop=True)
            rz = spool.tile([128, 28], BF16, tag="vrz")
            nc.scalar.activation(rz[:], prz[:],
                                 mybir.ActivationFunctionType.Sigmoid)
            u = spool.tile([128, 14], BF16, tag="vu")
            nc.vector.tensor_tensor(u[:], pn[:], cb["vbhn_bc"][:],
                                    op=mybir.AluOpType.add)
            nc.vector.tensor_tensor(u[:], rz[:, 0:14], u[:],
                                    op=mybir.AluOpType.mult)
            nc.vector.tensor_tensor(
                u[:], u[:],
                vxg[:].rearrange("p (g k b v2) -> p g k b v2", g=3, k=7, b=BL)
                [:, 2, :, :, v],
                op=mybir.AluOpType.add)
            nt = spool.tile([128, 14], BF16, tag="vnt")
            nc.scalar.activation(nt[:], u[:],
                                 mybir.ActivationFunctionType.Tanh)
            t3 = spool.tile([128, 14], BF16, tag="vt3")
            nc.vector.tensor_tensor(t3[:], vh_prev[:], nt[:],
                                    op=mybir.AluOpType.subtract)
            nc.vector.tensor_tensor(t3[:], t3[:], rz[:, 14:28],
                                    op=mybir.AluOpType.mult)
            vh_new = hpool.tile([128, 14], BF16, tag="VH")
            nc.vector.tensor_tensor(vh_new[:], t3[:], nt[:],
                                    op=mybir.AluOpType.add)
            vh_prev = vh_new

        # ---- FC head
        rl = spool.tile([128, 14], BF16, tag="rl")
        nc.scalar.activation(rl[:], vh_prev[:],
                             mybir.ActivationFunctionType.Relu)
        pfc = ppool.tile([BL, OUT], F32, tag="pn")
        for k in range(7):
            nc.tensor.matmul(pfc[:], rl[:, k * 2:(k + 1) * 2],
                             cb["fcw"][:, k * OUT:(k + 1) * OUT],
                             start=(k == 0), stop=False)
        nc.tensor.matmul(pfc[:], ones[0:1, 0:BL], cb["fcb"][0:1, :],
                         start=False, stop=True)
        lg = spool.tile([BL, OUT], F32, tag="lg")
        nc.scalar.copy(lg[:], pfc[:])
        nc.sync.dma_start(out_logits.ap(), lg[:])


# --------------------------------------------------------------------------
# entry point
# --------------------------------------------------------------------------

_NC_CACHE = None


def kernel(**inputs):
    global _NC_CACHE
    if _NC_CACHE is None:
        _NC_CACHE = build_nc()
    nc = _NC_CACHE
    shared = _prep_shared(inputs)
    in_maps = [_prep_core(inputs, shared, c) for c in range(NCORES)]
    res = run_bass_kernel_spmd(nc, in_maps, core_ids=list(range(NCORES)))
    return np.concatenate([res.results[c]["logits"] for c in range(NCORES)],
                          axis=0).astype(np.float32)


if __name__ == "__main__":
    import reference
    inputs = {k: np.asarray(v) for k, v in reference.setup_inputs().items()}
    out = kernel(**inputs)
    print("out", out.shape, out.dtype)
